# revision 9
# baseline (speedup 1.0000x reference)
"""Trainium2 Bass kernel for nn_Attention_53687091200195.

Reference computation (per batch b):
    Q = relu(x @ Wq + bq); K = relu(x @ Wk + bk); V = relu(x @ Wv + bv)
    S = Q @ K^T / sqrt(64); P = softmax(S, axis=-1); out = P @ V

Shapes: x [16, 2048, 64] f32, W* [64, 128] f32, b* [128] f32 -> out [16, 2048, 128].

Sharding: data-parallel over batch. 8 cores x 2 batches each; weights replicated.

Per-core design (SPMD, identical program):
  - Token-permuted layout: internal token index n~ = j*128 + p maps to real token
    p*16 + j.  Attention is permutation-equivariant over tokens, so computing on
    permuted tokens and writing output through the inverse permutation is exact,
    and it makes the x-load / out-store DMAs contiguous per partition.
  - fp8(e4m3) DoubleRow matmuls: the PE streams 2 fp8 columns/cycle and sums a
    [p, 2, *]-paired 2x128 contraction per instruction.  Scores contract d=128
    as [64, 2] (Q/K kept in a d-split layout qT2[p, i, n] = Q[n, i*64+p]);
    PV and the denominator contract keys as adjacent m-tile pairs of the
    natural e_all[key, m, n] / v_sb[key, m, d] layouts.  Softmax weights
    tolerate fp8: weight-noise averages out over ~2048 keys and the
    denominator (computed from the same fp8 E) cancels the common mode.
  - exp (8.4M elements/core) is split across the two PSUM-capable vector
    engines per a fixed per-m map: ACT tiles use the exact table Exp with fp8
    output; DVE tiles use a one-pass Schraudolph: fp8e4m3 bits =
    int8(S*1.4427 + B), i.e. exp(S/8) within +-4% -- noise that averages out
    over keys, and whose common mode cancels num/den.  (GPSIMD cannot access
    PSUM on TRN2, so it only handles memsets and DMA descriptors.)
  - Projections contract c=65 (bias folded as an all-ones xT row), xT/weights
    in bf16.  Q/K projections write the d-split PSUM tile [64, 2, n] directly
    (two d-half matmuls per 512-token slice); the relu + fp8-quantize
    PSUM->SBUF pass alternates ACT/DVE.  V tiles [m, d] are computed directly
    (xT_j stationary, Wv moving), batched 4 token-tiles per PSUM tile.
  - Per 1024-query chunk: S^T = K_m Q^T (fp8 DoubleRow over d), E = exp tiles,
    outT += V_pair^T E_pair, den += ones^T E_pair (fp8 DoubleRow over 2x128
    keys).  PV lags exp by ~2 m-iters, den by 4; the denominator reciprocal
    is ready before the epilogue out-transposes (bf16 data), so the
    normalize is fused into the single PSUM->SBUF copy per output tile
    (tensor_scalar mult by 1/den).  Stores run in halves so the first DMA
    overlaps the second half's work.
"""

import numpy as np

import concourse.bass as bass
import concourse.mybir as mybir
import concourse.tile as tile
from concourse import bacc
from concourse.bass_utils import run_bass_kernel_spmd

N_CORES = 8
B_PER_CORE = 2
N_TOK = 2048
C_IN = 64
D = 128
P = 128
N_TILES = N_TOK // P          # 16
N_CHUNK = 1024
N_CHUNKS = N_TOK // N_CHUNK   # 2
JT = N_CHUNK // P             # 8
SCALE = 1.0 / 8.0             # 1/sqrt(64)

F32 = mybir.dt.float32
BF16 = mybir.dt.bfloat16
FP8 = mybir.dt.float8e4
I8 = mybir.dt.int8

# Schraudolph constants for exp(S/8) producing fp8e4m3 bits via int8:
# bits = S * (8*log2e/8) + 8*7 + corrections (+0.5 for the truncating
# f32->int8 convert, -0.344 to center the piecewise-linear 2^frac sawtooth).
SCH_A = 1.4426950408889634
SCH_B = 56.0 + 0.5 - 0.5 * 8.0 * 0.0861

# exp engine per m-iter (period 16): A=ACT exact-table, D=DVE schraudolph.
EXP_PATTERN = "ADADADADAADAADAA"
# relu engine for the 16 q/k projection slices
RELU_PATTERN = "ADADADADADADADAD"


def build_program():
    nc = bacc.Bacc("TRN2", target_bir_lowering=False, debug=False,
                   num_devices=N_CORES)

    x = nc.dram_tensor("x", [B_PER_CORE, N_TOK, C_IN], F32, kind="ExternalInput").ap()
    wq = nc.dram_tensor("Wq", [C_IN, D], F32, kind="ExternalInput").ap()
    bq = nc.dram_tensor("bq", [D], F32, kind="ExternalInput").ap()
    wk = nc.dram_tensor("Wk", [C_IN, D], F32, kind="ExternalInput").ap()
    bk = nc.dram_tensor("bk", [D], F32, kind="ExternalInput").ap()
    wv = nc.dram_tensor("Wv", [C_IN, D], F32, kind="ExternalInput").ap()
    bv = nc.dram_tensor("bv", [D], F32, kind="ExternalInput").ap()
    out = nc.dram_tensor("out", [B_PER_CORE, N_TOK, D], F32, kind="ExternalOutput").ap()

    with tile.TileContext(nc) as tc:
        kernel_body(tc, out, x, (wq, bq), (wk, bk), (wv, bv))

    nc.compile()
    return nc


def kernel_body(tc, out, x, qw, kw, vw):
    nc = tc.nc
    from contextlib import ExitStack
    ctx = ExitStack()
    with ctx:
        consts = ctx.enter_context(tc.tile_pool(name="consts", bufs=1))
        perb = ctx.enter_context(tc.tile_pool(name="perb", bufs=2))
        epool = ctx.enter_context(tc.tile_pool(name="epool", bufs=1))
        ep = ctx.enter_context(tc.tile_pool(name="ep", bufs=2))

        # --- constants ---
        identity = consts.tile([P, P], F32)
        nc.vector.memset(identity[:], 0.0)
        nc.gpsimd.affine_select(
            out=identity[:], in_=identity[:],
            compare_op=mybir.AluOpType.not_equal, fill=1.0,
            base=0, pattern=[[-1, P]], channel_multiplier=1)
        id16 = consts.tile([P, P], BF16)
        nc.vector.tensor_copy(out=id16[:], in_=identity[:])
        # [P, 2, 16] so the DoubleRow lhsT outermost free step is 16B-aligned
        ones8 = consts.tile([P, 2, 16], FP8)
        nc.vector.memset(ones8[:], 1.0)

        # x for both batches, token-permuted: x_nat2[p, j, b, c] = x[b, p*16+j, c].
        x_nat2 = consts.tile([P, N_TILES, B_PER_CORE, C_IN], F32, name="x_nat2",
                             tag="x_nat2")
        H = N_TILES // 4
        for jh in range(4):
            for bb in range(B_PER_CORE):
                eng = nc.sync if bb == 0 else nc.gpsimd
                eng.dma_start(
                    out=x_nat2[:, jh * H:(jh + 1) * H, bb, :],
                    in_=bass.AP(
                        tensor=x.tensor,
                        offset=bb * N_TOK * C_IN + jh * H * C_IN,
                        ap=[[N_TILES * C_IN, P], [C_IN, H], [1, C_IN]],
                    ),
                )

        # Bias-folded weights, bf16, with the d-split for q/k:
        # w2[c, dh, dl] = W[c, dh*64+dl] (c=64 row is the bias).
        w_sb = {}
        for name, (w, b) in (("q", qw), ("k", kw), ("v", vw)):
            wf = consts.tile([C_IN, D], F32, name=f"wf_{name}", tag=f"wf_{name}")
            nc.sync.dma_start(out=wf[:], in_=w[:])
            bf = consts.tile([1, D], F32, name=f"bf_{name}", tag=f"bf_{name}")
            nc.sync.dma_start(out=bf[:], in_=b[:])
            w2 = consts.tile([C_IN + 1, 2, C_IN], BF16, name=f"w_{name}",
                             tag=f"w_{name}")
            nc.vector.tensor_copy(out=w2[0:C_IN, :, :],
                                  in_=wf[:].rearrange("p (h d) -> p h d", h=2))
            nc.vector.tensor_copy(out=w2[C_IN:C_IN + 1, :, :],
                                  in_=bf[:].rearrange("p (h d) -> p h d", h=2))
            w_sb[name] = w2

        xTs = [perb.tile([C_IN + 1, N_TOK], BF16, name=f"xT_{bb}",
                         tag=f"xT_{bb}", bufs=1)
               for bb in range(B_PER_CORE)]
        for bb in range(B_PER_CORE):
            nc.gpsimd.memset(xTs[bb][C_IN:C_IN + 1, :], 1.0)

        # d-split fp8 Q/K: qT2[p, i, n] = Q[n, i*64+p]
        qT2s = [perb.tile([C_IN, 2, N_TOK], FP8, name=f"qT2_{bb}",
                          tag=f"qT2_{bb}", bufs=1) for bb in range(B_PER_CORE)]
        kT2s = [perb.tile([C_IN, 2, N_TOK], FP8, name=f"kT2_{bb}",
                          tag=f"kT2_{bb}", bufs=1) for bb in range(B_PER_CORE)]
        # v_sb[p, m, d] = V[m*128+p, d], fp8
        v_sbs = [perb.tile([P, N_TILES, D], FP8, name=f"v_sb_{bb}",
                           tag=f"v_sb_{bb}", bufs=1) for bb in range(B_PER_CORE)]

        relu_iter = list(RELU_PATTERN)

        # ---------------- Phase A: prologue (own PSUM scope) ----------------
        with tc.tile_pool(name="ptr", bufs=2, space="PSUM") as ptr, \
             tc.tile_pool(name="pvp", bufs=2, space="PSUM") as pvp, \
             tc.tile_pool(name="ppj", bufs=2, space="PSUM") as ppj:

            def x_tr(q):
                # transposes for j = 4q..4q+3 -> one PSUM tile, 2 copies
                xt4 = ptr.tile([P, 4, P], F32, tag="tr", name=f"xt4_{q}")
                for js in range(4):
                    j = 4 * q + js
                    nc.tensor.transpose(
                        xt4[:, js, :], x_nat2[:, j, :, :], identity[:])
                for bb in range(B_PER_CORE):
                    src = xt4[bb * C_IN:(bb + 1) * C_IN, :, :]
                    dst = xTs[bb][0:C_IN, 4 * q * P:(4 * q + 4) * P]
                    if bb == 0:
                        nc.vector.tensor_copy(
                            out=dst.rearrange("c (j t) -> c j t", j=4), in_=src)
                    else:
                        nc.scalar.copy(
                            out=dst.rearrange("c (j t) -> c j t", j=4), in_=src)

            def v_dir(bb, q):
                # V for token tiles 4q..4q+3 of batch bb
                vp = pvp.tile([P, 4, P], F32, tag="vp", name=f"vp_{bb}_{q}")
                for js in range(4):
                    j = 4 * q + js
                    nc.tensor.matmul(
                        vp[:, js, :],
                        xTs[bb][:, j * P:(j + 1) * P],
                        w_sb["v"][:].rearrange("p h d -> p (h d)"),
                        start=True, stop=True)
                dst = v_sbs[bb][:, 4 * q:4 * q + 4, :]
                if bb == 0:
                    nc.vector.tensor_scalar_max(dst, vp[:], 0.0)
                else:
                    nc.scalar.activation(
                        out=dst, in_=vp[:],
                        func=mybir.ActivationFunctionType.Relu, scale=1.0)

            def qk_unit(bb, name, s):
                # projection slice s (512 tokens), both d-halves
                t = (qT2s if name == "q" else kT2s)[bb]
                pj = ppj.tile([C_IN, 2, 512], F32, tag="pj",
                              name=f"pj_{bb}_{name}_{s}")
                for dh in range(2):
                    nc.tensor.matmul(
                        pj[:, dh, :], w_sb[name][:, dh, :],
                        xTs[bb][:, s * 512:(s + 1) * 512],
                        start=True, stop=True)
                eng = relu_iter.pop(0)
                dst = t[:, :, s * 512:(s + 1) * 512]
                if eng == "A":
                    nc.scalar.activation(
                        out=dst, in_=pj[:],
                        func=mybir.ActivationFunctionType.Relu, scale=1.0)
                else:
                    nc.vector.tensor_scalar_max(dst, pj[:], 0.0)

            # schedule: x-transpose quads; V quads and q/k projection slices
            # fill in one quad behind
            for q in range(4):
                x_tr(q)
                if q >= 1:
                    for bb in range(B_PER_CORE):
                        v_dir(bb, q - 1)
                    s = q - 1
                    for name in ("q", "k"):
                        for bb in range(B_PER_CORE):
                            qk_unit(bb, name, s)
            for bb in range(B_PER_CORE):
                v_dir(bb, 3)
            for name in ("q", "k"):
                for bb in range(B_PER_CORE):
                    qk_unit(bb, name, 3)

        # ---------------- Phase B: attention sweeps ----------------
        pst = ctx.enter_context(tc.tile_pool(name="pst", bufs=2, space="PSUM"))
        pacc = ctx.enter_context(tc.tile_pool(name="pacc", bufs=1, space="PSUM"))
        pden = ctx.enter_context(tc.tile_pool(name="pden", bufs=1, space="PSUM"))

        e_all = epool.tile([P, N_TILES, N_CHUNK], FP8, tag="e_all",
                           name="e_all")
        exp_engs = list(EXP_PATTERN)

        for b in range(B_PER_CORE):
            qT2, kT2, v_sb = qT2s[b], kT2s[b], v_sbs[b]
            for chunk in range(N_CHUNKS):
                n0 = chunk * N_CHUNK
                acc = pacc.tile([P, N_CHUNK], F32, tag="acc",
                                name=f"acc_{b}_{chunk}")
                den = pden.tile([1, N_CHUNK], F32, tag="den",
                                name=f"den_{b}_{chunk}")
                outu = ep.tile([P, N_CHUNK], BF16, tag="outu",
                               name=f"outu_{b}_{chunk}")
                o_sb = ep.tile([P, JT, D], F32, tag="o_sb",
                               name=f"o_sb_{b}_{chunk}")
                den_sb = ep.tile([1, N_CHUNK], F32, tag="den_sb",
                                 name=f"den_sb_{b}_{chunk}")
                recip = ep.tile([P, JT], F32, tag="recip",
                                name=f"recip_{b}_{chunk}")

                for m in range(N_TILES + 8):
                    if m < N_TILES:
                        st = pst.tile([P, N_CHUNK], F32, tag="st",
                                      name=f"st_{b}_{chunk}_{m}")
                        for h in range(2):
                            nc.tensor.matmul(
                                st[:, h * 512:(h + 1) * 512],
                                kT2[:, :, m * P:(m + 1) * P],
                                qT2[:, :, n0 + h * 512:n0 + (h + 1) * 512],
                                start=True, stop=True,
                                perf_mode=mybir.MatmulPerfMode.DoubleRow)
                        if exp_engs[m] == "A":
                            nc.scalar.activation(
                                out=e_all[:, m, :], in_=st[:],
                                func=mybir.ActivationFunctionType.Exp,
                                scale=SCALE)
                        else:
                            nc.vector.tensor_scalar(
                                out=e_all[:, m, :].bitcast(I8), in0=st[:],
                                scalar1=SCH_A, scalar2=SCH_B,
                                op0=mybir.AluOpType.mult,
                                op1=mybir.AluOpType.add)
                    # PV: pair a at iter 2a+3
                    if m >= 3 and m % 2 == 1 and (m - 3) // 2 < N_TILES // 2:
                        a = (m - 3) // 2
                        for h in range(2):
                            nc.tensor.matmul(
                                acc[:, h * 512:(h + 1) * 512],
                                v_sb[:, 2 * a:2 * a + 2, :],
                                e_all[:, 2 * a:2 * a + 2,
                                      h * 512:(h + 1) * 512],
                                start=(a == 0), stop=(a == N_TILES // 2 - 1),
                                perf_mode=mybir.MatmulPerfMode.DoubleRow)
                        if a == N_TILES // 2 - 1:
                            nc.vector.tensor_copy(out=outu[:], in_=acc[:])
                    # den: pair a at iter 2a+5; reciprocal chain right after
                    if m >= 5 and m % 2 == 1 and (m - 5) // 2 < N_TILES // 2:
                        a = (m - 5) // 2
                        for h in range(2):
                            nc.tensor.matmul(
                                den[:, h * 512:(h + 1) * 512],
                                ones8[:, :, 0:1],
                                e_all[:, 2 * a:2 * a + 2,
                                      h * 512:(h + 1) * 512],
                                start=(a == 0), stop=(a == N_TILES // 2 - 1),
                                perf_mode=mybir.MatmulPerfMode.DoubleRow)
                        if a == N_TILES // 2 - 1:
                            nc.scalar.copy(out=den_sb[:], in_=den[:])
                    if m == N_TILES + 3:
                        den_t = pst.tile([P, JT], F32, tag="st",
                                         name=f"den_t_{b}_{chunk}")
                        for jt in range(JT):
                            nc.tensor.transpose(den_t[:, jt:jt + 1],
                                                den_sb[:, jt * P:(jt + 1) * P],
                                                identity[:1, :1])
                        nc.vector.reciprocal(out=recip[:], in_=den_t[:])
                    # out-transposes (iters 20..23) with the normalize fused
                    # into each PSUM->SBUF copy
                    if N_TILES + 4 <= m < N_TILES + 8:
                        for jt in range((m - N_TILES - 4) * 2,
                                        (m - N_TILES - 3) * 2):
                            tr_ps = pst.tile([P, P], BF16, tag="st",
                                             name=f"tr_{b}_{chunk}_{jt}")
                            nc.tensor.transpose(tr_ps[:],
                                                outu[:, jt * P:(jt + 1) * P],
                                                id16[:])
                            nc.vector.tensor_scalar(
                                out=o_sb[:, jt, :], in0=tr_ps[:],
                                scalar1=recip[:, jt:jt + 1], scalar2=None,
                                op0=mybir.AluOpType.mult)
                        if m == N_TILES + 5 or m == N_TILES + 7:
                            half = (m - N_TILES - 5) // 2
                            nc.sync.dma_start(
                                out=bass.AP(
                                    tensor=out.tensor,
                                    offset=(b * N_TOK + chunk * JT
                                            + half * JT // 2) * D,
                                    ap=[[N_TILES * D, P], [D, JT // 2], [1, D]],
                                ),
                                in_=o_sb[:, half * JT // 2:(half + 1) * JT // 2, :],
                            )


_NC_CACHE = None


def _get_program():
    global _NC_CACHE
    if _NC_CACHE is None:
        _NC_CACHE = build_program()
    return _NC_CACHE


def kernel(x, Wq, bq, Wk, bk, Wv, bv, _trace=False):
    x = np.ascontiguousarray(np.asarray(x, dtype=np.float32))
    full_b = x.shape[0]
    assert full_b == N_CORES * B_PER_CORE, x.shape
    nc = _get_program()
    common = {
        "Wq": np.ascontiguousarray(np.asarray(Wq, np.float32)),
        "bq": np.ascontiguousarray(np.asarray(bq, np.float32)),
        "Wk": np.ascontiguousarray(np.asarray(Wk, np.float32)),
        "bk": np.ascontiguousarray(np.asarray(bk, np.float32)),
        "Wv": np.ascontiguousarray(np.asarray(Wv, np.float32)),
        "bv": np.ascontiguousarray(np.asarray(bv, np.float32)),
    }
    in_maps = [
        {"x": x[c * B_PER_CORE:(c + 1) * B_PER_CORE], **common}
        for c in range(N_CORES)
    ]
    res = run_bass_kernel_spmd(nc, in_maps, list(range(N_CORES)), trace=_trace)
    outs = np.concatenate([res.results[c]["out"] for c in range(N_CORES)], axis=0)
    if _trace:
        kernel.last_exec_time_ns = res.exec_time_ns
        kernel.last_trace_info = (res.profile_json,
                                  (res.instructions_and_trace or (None, None))[1])
    return outs


# revision 11
# speedup vs baseline: 1.6081x; 1.6081x over previous
"""Trainium2 Bass kernel for nn_Attention_53687091200195.

Reference computation (per batch b):
    Q = relu(x @ Wq + bq); K = relu(x @ Wk + bk); V = relu(x @ Wv + bv)
    S = Q @ K^T / sqrt(64); P = softmax(S, axis=-1); out = P @ V

Shapes: x [16, 2048, 64] f32, W* [64, 128] f32, b* [128] f32 -> out [16, 2048, 128].

Sharding: data-parallel over batch. 8 cores x 2 batches each; weights replicated.

Measured TRN2 facts this design is built on (from NTFF traces of prior
versions): the PE streams 1 moving-column/cycle at 2.4GHz for every dtype
(fp32r/bf16/fp8; fp8 DoubleRow only fuses instructions, no throughput gain),
a 512-col matmul slot is ~245ns end-to-end; ACT costs ~1.0-1.35ns/element
(narrow output dtypes are slower); DVE ~1.3-1.5ns/element for fp32-PSUM
input ops, with 2x/4x packing modes possible only for all-SBUF 2-byte ops.

Per-core design (SPMD, identical program):
  - Token-permuted layout: internal token n~ = j*128 + p maps to real token
    p*16 + j; attention is permutation-equivariant, and this makes the
    x-load / out-store DMAs contiguous per partition.
  - All-bf16 operand path (same PE speed as fp32r, half the SBUF traffic):
    xT [65, 2048] (bias folded as ones row), Q^T/K^T [128d, 2048] bf16,
    V [key, m, d] bf16, E [key, m, n] bf16.
  - The denominator is NOT a PE stream: esum[p, n] = sum_m E[p, m, n] is a
    bf16 DVE add-tree (two half-trees of fat 4096/2048/1024-elem adds,
    scheduled as the exps complete), then one [1, n] ones-matmul pair +
    transposed reciprocal.  The per-element tree rounding noise averages
    out 1/sqrt(128) in the partition sum.  This removes one of the three
    full E-sized PE streams (~31us) for ~7us of DVE work.
  - exp engine split per m (EXP_PATTERN): ACT exact-table Exp (bf16 out) by
    default, DVE one-pass Schraudolph for relief tiles: bf16 bits =
    int16(S*23.083 + 16251) == exp(S/8) within +-4% -- per-key noise that
    averages out in PV, common mode cancelled by the denominator.
  - Cross-chunk software pipelining: each (batch, chunk) window runs a
    26-iteration schedule (scores+exp 0-15, PV lag 2, esum trees 11-13 and
    17-19, den matmul 20, reciprocal 22, out-transposes 21-24 with the
    normalize fused into the PSUM->SBUF copy, stores 24/25), and windows
    start every 18 iterations, so one window's epilogue tail executes under
    the next window's score stream and the PE never drains between chunks.
  - PSUM: score ring 2x4KB, PV accumulator 4KB, small pool for the output
    transposes + den (2KB, h-halves sequential) -- the score ring keeps a
    pure 1-tile/iter cadence so its WAR chase stays exactly 2 exps behind.
"""

import numpy as np

import concourse.bass as bass
import concourse.mybir as mybir
import concourse.tile as tile
from concourse import bacc
from concourse.bass_utils import run_bass_kernel_spmd

N_CORES = 8
B_PER_CORE = 2
N_TOK = 2048
C_IN = 64
D = 128
P = 128
N_TILES = N_TOK // P          # 16
N_CHUNK = 1024
N_CHUNKS = N_TOK // N_CHUNK   # 2
JT = N_CHUNK // P             # 8
SCALE = 1.0 / 8.0             # 1/sqrt(64)

F32 = mybir.dt.float32
BF16 = mybir.dt.bfloat16
I16 = mybir.dt.int16

# Schraudolph constants for exp(S/8) as bf16 bits via int16:
# bits = S * (128*log2e/8) + 128*127 + corrections (+0.5 truncating
# convert, -5.51 centers the piecewise-linear 2^frac sawtooth).
SCH_A = 23.083120654223414
SCH_B = 16256.0 + 0.5 - 0.5 * 128.0 * 0.0861

# exp engine per m (A=ACT exact, D=DVE schraudolph)
EXP_PATTERN = "AADAAADAAADAAADA"
# relu engine for the 8 projection slices
RELU_PATTERN = "ADADADAD"

WLEN = 26      # window schedule length (iters)
STRIDE = 18    # window start spacing


def build_program():
    nc = bacc.Bacc("TRN2", target_bir_lowering=False, debug=False,
                   num_devices=N_CORES)

    x = nc.dram_tensor("x", [B_PER_CORE, N_TOK, C_IN], F32, kind="ExternalInput").ap()
    wq = nc.dram_tensor("Wq", [C_IN, D], F32, kind="ExternalInput").ap()
    bq = nc.dram_tensor("bq", [D], F32, kind="ExternalInput").ap()
    wk = nc.dram_tensor("Wk", [C_IN, D], F32, kind="ExternalInput").ap()
    bk = nc.dram_tensor("bk", [D], F32, kind="ExternalInput").ap()
    wv = nc.dram_tensor("Wv", [C_IN, D], F32, kind="ExternalInput").ap()
    bv = nc.dram_tensor("bv", [D], F32, kind="ExternalInput").ap()
    out = nc.dram_tensor("out", [B_PER_CORE, N_TOK, D], F32, kind="ExternalOutput").ap()

    with tile.TileContext(nc) as tc:
        kernel_body(tc, out, x, (wq, bq), (wk, bk), (wv, bv))

    nc.compile()
    return nc


def kernel_body(tc, out, x, qw, kw, vw):
    nc = tc.nc
    from contextlib import ExitStack
    ctx = ExitStack()
    with ctx:
        consts = ctx.enter_context(tc.tile_pool(name="consts", bufs=1))
        perb = ctx.enter_context(tc.tile_pool(name="perb", bufs=2))
        epool = ctx.enter_context(tc.tile_pool(name="epool", bufs=1))
        ep = ctx.enter_context(tc.tile_pool(name="ep", bufs=2))

        # --- constants ---
        identity = consts.tile([P, P], F32)
        nc.vector.memset(identity[:], 0.0)
        nc.gpsimd.affine_select(
            out=identity[:], in_=identity[:],
            compare_op=mybir.AluOpType.not_equal, fill=1.0,
            base=0, pattern=[[-1, P]], channel_multiplier=1)
        id16 = consts.tile([P, P], BF16)
        nc.vector.tensor_copy(out=id16[:], in_=identity[:])
        ones16 = consts.tile([P, 16], BF16)
        nc.vector.memset(ones16[:], 1.0)

        # x for both batches, token-permuted: x_nat2[p, j, b, c] = x[b, p*16+j, c]
        x_nat2 = consts.tile([P, N_TILES, B_PER_CORE, C_IN], F32, name="x_nat2",
                             tag="x_nat2")
        H = N_TILES // 4
        for jh in range(4):
            for bb in range(B_PER_CORE):
                eng = nc.sync if bb == 0 else nc.gpsimd
                eng.dma_start(
                    out=x_nat2[:, jh * H:(jh + 1) * H, bb, :],
                    in_=bass.AP(
                        tensor=x.tensor,
                        offset=bb * N_TOK * C_IN + jh * H * C_IN,
                        ap=[[N_TILES * C_IN, P], [C_IN, H], [1, C_IN]],
                    ),
                )

        # Bias-folded weights in bf16: w2[c, d], c=64 row is the bias.
        w_sb = {}
        for name, (w, b) in (("q", qw), ("k", kw), ("v", vw)):
            wf = consts.tile([C_IN, D], F32, name=f"wf_{name}", tag=f"wf_{name}")
            nc.sync.dma_start(out=wf[:], in_=w[:])
            bf = consts.tile([1, D], F32, name=f"bf_{name}", tag=f"bf_{name}")
            nc.sync.dma_start(out=bf[:], in_=b[:])
            w2 = consts.tile([C_IN + 1, D], BF16, name=f"w_{name}",
                             tag=f"w_{name}")
            nc.vector.tensor_copy(out=w2[0:C_IN, :], in_=wf[:])
            nc.vector.tensor_copy(out=w2[C_IN:C_IN + 1, :], in_=bf[:])
            w_sb[name] = w2

        xTs = [perb.tile([C_IN + 1, N_TOK], BF16, name=f"xT_{bb}",
                         tag=f"xT_{bb}", bufs=1)
               for bb in range(B_PER_CORE)]
        for bb in range(B_PER_CORE):
            nc.gpsimd.memset(xTs[bb][C_IN:C_IN + 1, :], 1.0)

        qTs = [perb.tile([D, N_TOK], BF16, name=f"qT_{bb}",
                         tag=f"qT_{bb}", bufs=1) for bb in range(B_PER_CORE)]
        kTs = [perb.tile([D, N_TOK], BF16, name=f"kT_{bb}",
                         tag=f"kT_{bb}", bufs=1) for bb in range(B_PER_CORE)]
        v_sbs = [perb.tile([P, N_TILES, D], BF16, name=f"v_sb_{bb}",
                           tag=f"v_sb_{bb}", bufs=1) for bb in range(B_PER_CORE)]

        relu_iter = list(RELU_PATTERN)

        # ---------------- Phase A: prologue (own PSUM scope) ----------------
        with tc.tile_pool(name="ptr", bufs=2, space="PSUM") as ptr, \
             tc.tile_pool(name="pvp", bufs=2, space="PSUM") as pvp, \
             tc.tile_pool(name="ppj", bufs=2, space="PSUM") as ppj:

            def x_tr(q):
                xt4 = ptr.tile([P, 4, P], F32, tag="tr", name=f"xt4_{q}")
                for js in range(4):
                    j = 4 * q + js
                    nc.tensor.transpose(
                        xt4[:, js, :], x_nat2[:, j, :, :], identity[:])
                for bb in range(B_PER_CORE):
                    src = xt4[bb * C_IN:(bb + 1) * C_IN, :, :]
                    dst = xTs[bb][0:C_IN, 4 * q * P:(4 * q + 4) * P]
                    if bb == 0:
                        nc.vector.tensor_copy(
                            out=dst.rearrange("c (j t) -> c j t", j=4), in_=src)
                    else:
                        nc.scalar.copy(
                            out=dst.rearrange("c (j t) -> c j t", j=4), in_=src)

            def v_dir(bb, q):
                vp = pvp.tile([P, 4, P], F32, tag="vp", name=f"vp_{bb}_{q}")
                for js in range(4):
                    j = 4 * q + js
                    nc.tensor.matmul(
                        vp[:, js, :],
                        xTs[bb][:, j * P:(j + 1) * P],
                        w_sb["v"][:],
                        start=True, stop=True)
                dst = v_sbs[bb][:, 4 * q:4 * q + 4, :]
                if bb == 0:
                    nc.vector.tensor_scalar_max(dst, vp[:], 0.0)
                else:
                    nc.scalar.activation(
                        out=dst, in_=vp[:],
                        func=mybir.ActivationFunctionType.Relu, scale=1.0)

            def qk_unit(bb, name, s):
                # projection slice s (1024 tokens)
                t = (qTs if name == "q" else kTs)[bb]
                pj = ppj.tile([D, 2, 512], F32, tag="pj",
                              name=f"pj_{bb}_{name}_{s}")
                for h in range(2):
                    nc.tensor.matmul(
                        pj[:, h, :], w_sb[name][:],
                        xTs[bb][:, s * 1024 + h * 512:s * 1024 + (h + 1) * 512],
                        start=True, stop=True)
                eng = relu_iter.pop(0)
                dst = t[:, s * 1024:(s + 1) * 1024]
                if eng == "A":
                    nc.scalar.activation(
                        out=dst, in_=pj[:],
                        func=mybir.ActivationFunctionType.Relu, scale=1.0)
                else:
                    nc.vector.tensor_scalar_max(dst, pj[:], 0.0)

            for q in range(4):
                x_tr(q)
                if q >= 1:
                    for bb in range(B_PER_CORE):
                        v_dir(bb, q - 1)
                if q == 2:
                    for name in ("q", "k"):
                        for bb in range(B_PER_CORE):
                            qk_unit(bb, name, 0)
            for bb in range(B_PER_CORE):
                v_dir(bb, 3)
            for name in ("q", "k"):
                for bb in range(B_PER_CORE):
                    qk_unit(bb, name, 1)

        # ---------------- Phase B: pipelined attention windows ----------------
        pst = ctx.enter_context(tc.tile_pool(name="pst", bufs=2, space="PSUM"))
        pacc = ctx.enter_context(tc.tile_pool(name="pacc", bufs=1, space="PSUM"))
        ptr2 = ctx.enter_context(tc.tile_pool(name="ptr2", bufs=2, space="PSUM"))

        e_all = epool.tile([P, N_TILES, N_CHUNK], BF16, tag="e_all",
                           name="e_all")
        tmpA = epool.tile([P, 4, N_CHUNK], BF16, tag="tmpA", name="tmpA")
        tmpB = epool.tile([P, 4, N_CHUNK], BF16, tag="tmpB", name="tmpB")
        exp_engs = list(EXP_PATTERN)

        windows = [(b, c) for b in range(B_PER_CORE) for c in range(N_CHUNKS)]
        state = {}

        def emit(wi, t):
            b, chunk = windows[wi]
            qT, kT, v_sb = qTs[b], kTs[b], v_sbs[b]
            n0 = chunk * N_CHUNK
            if t == 0:
                state[wi] = {
                    "acc": pacc.tile([P, N_CHUNK], F32, tag="acc",
                                     name=f"acc_{wi}"),
                    "outu": ep.tile([P, N_CHUNK], F32, tag="outu",
                                    name=f"outu_{wi}"),
                    "o_sb": ep.tile([P, JT, D], F32, tag="o_sb",
                                    name=f"o_sb_{wi}"),
                    "den_sb": ep.tile([1, N_CHUNK], F32, tag="den_sb",
                                      name=f"den_sb_{wi}"),
                    "recip": ep.tile([P, JT], F32, tag="recip",
                                     name=f"recip_{wi}"),
                }
            st_ = state[wi]

            if t < N_TILES:
                m = t
                st = pst.tile([P, N_CHUNK], F32, tag="st",
                              name=f"st_{wi}_{m}")
                for h in range(2):
                    nc.tensor.matmul(
                        st[:, h * 512:(h + 1) * 512],
                        kT[:, m * P:(m + 1) * P],
                        qT[:, n0 + h * 512:n0 + (h + 1) * 512],
                        start=True, stop=True)
                if exp_engs[m] == "A":
                    nc.scalar.activation(
                        out=e_all[:, m, :], in_=st[:],
                        func=mybir.ActivationFunctionType.Exp, scale=SCALE)
                else:
                    nc.vector.tensor_scalar(
                        out=e_all[:, m, :].bitcast(I16), in0=st[:],
                        scalar1=SCH_A, scalar2=SCH_B,
                        op0=mybir.AluOpType.mult,
                        op1=mybir.AluOpType.add)
            # PV lag 2
            if 2 <= t < N_TILES + 2:
                m = t - 2
                for h in range(2):
                    nc.tensor.matmul(
                        st_["acc"][:, h * 512:(h + 1) * 512],
                        v_sb[:, m, :],
                        e_all[:, m, h * 512:(h + 1) * 512],
                        start=(m == 0), stop=(m == N_TILES - 1))
                if m == N_TILES - 1:
                    nc.vector.tensor_copy(out=st_["outu"][:], in_=st_["acc"][:])
            # esum half-trees (bf16, all-SBUF)
            if t == 11:
                nc.vector.tensor_tensor(
                    out=tmpA[:], in0=e_all[:, 0:4, :], in1=e_all[:, 4:8, :],
                    op=mybir.AluOpType.add)
            if t == 12:
                nc.vector.tensor_tensor(
                    out=tmpA[:, 0:2, :], in0=tmpA[:, 0:2, :],
                    in1=tmpA[:, 2:4, :], op=mybir.AluOpType.add)
            if t == 13:
                nc.vector.tensor_tensor(
                    out=tmpA[:, 0:1, :], in0=tmpA[:, 0:1, :],
                    in1=tmpA[:, 1:2, :], op=mybir.AluOpType.add)
            if t == 17:
                nc.vector.tensor_tensor(
                    out=tmpB[:], in0=e_all[:, 8:12, :], in1=e_all[:, 12:16, :],
                    op=mybir.AluOpType.add)
            if t == 18:
                nc.vector.tensor_tensor(
                    out=tmpB[:, 0:2, :], in0=tmpB[:, 0:2, :],
                    in1=tmpB[:, 2:4, :], op=mybir.AluOpType.add)
            if t == 19:
                nc.vector.tensor_tensor(
                    out=tmpB[:, 0:1, :], in0=tmpB[:, 0:1, :],
                    in1=tmpB[:, 1:2, :], op=mybir.AluOpType.add)
                nc.vector.tensor_tensor(
                    out=tmpA[:, 0:1, :], in0=tmpA[:, 0:1, :],
                    in1=tmpB[:, 0:1, :], op=mybir.AluOpType.add)
            if t == 20:
                # den = ones^T esum, h-halves into sequential 2KB tiles
                for h in range(2):
                    dn = ptr2.tile([1, 512], F32, tag="den",
                                   name=f"den_{wi}_{h}", bufs=1)
                    nc.tensor.matmul(
                        dn[:], ones16[:, 0:1],
                        tmpA[:, 0, h * 512:(h + 1) * 512],
                        start=True, stop=True)
                    nc.scalar.copy(out=st_["den_sb"][:, h * 512:(h + 1) * 512],
                                   in_=dn[:])
            if t == 21:
                # one scratch bank: 2-slot out-transpose ring + den_t columns
                scr = ptr2.tile([P, 2 * P + JT], F32, tag="scr",
                                name=f"scr_{wi}", bufs=1)
                st_["scr"] = scr
                for jt in range(JT):
                    nc.tensor.transpose(scr[:, 2 * P + jt:2 * P + jt + 1],
                                        st_["den_sb"][:, jt * P:(jt + 1) * P],
                                        identity[:1, :1])
                nc.vector.reciprocal(out=st_["recip"][:],
                                     in_=scr[:, 2 * P:2 * P + JT])
            # out-transposes (2/iter, t=21..24), normalize fused in the copy
            if 21 <= t < 25:
                for jt in range((t - 21) * 2, (t - 20) * 2):
                    slot = (jt % 2) * P
                    scr = st_["scr"]
                    nc.tensor.transpose(scr[:, slot:slot + P],
                                        st_["outu"][:, jt * P:(jt + 1) * P],
                                        identity[:])
                    nc.vector.tensor_scalar(
                        out=st_["o_sb"][:, jt, :], in0=scr[:, slot:slot + P],
                        scalar1=st_["recip"][:, jt:jt + 1], scalar2=None,
                        op0=mybir.AluOpType.mult)
            if t == 24 or t == 25:
                half = t - 24
                nc.sync.dma_start(
                    out=bass.AP(
                        tensor=out.tensor,
                        offset=(b * N_TOK + chunk * JT + half * JT // 2) * D,
                        ap=[[N_TILES * D, P], [D, JT // 2], [1, D]],
                    ),
                    in_=st_["o_sb"][:, half * JT // 2:(half + 1) * JT // 2, :],
                )

        total_g = STRIDE * (len(windows) - 1) + WLEN
        for g in range(total_g):
            for wi in range(len(windows)):
                t = g - STRIDE * wi
                if 0 <= t < WLEN:
                    emit(wi, t)


_NC_CACHE = None


def _get_program():
    global _NC_CACHE
    if _NC_CACHE is None:
        _NC_CACHE = build_program()
    return _NC_CACHE


def kernel(x, Wq, bq, Wk, bk, Wv, bv, _trace=False):
    x = np.ascontiguousarray(np.asarray(x, dtype=np.float32))
    full_b = x.shape[0]
    assert full_b == N_CORES * B_PER_CORE, x.shape
    nc = _get_program()
    common = {
        "Wq": np.ascontiguousarray(np.asarray(Wq, np.float32)),
        "bq": np.ascontiguousarray(np.asarray(bq, np.float32)),
        "Wk": np.ascontiguousarray(np.asarray(Wk, np.float32)),
        "bk": np.ascontiguousarray(np.asarray(bk, np.float32)),
        "Wv": np.ascontiguousarray(np.asarray(Wv, np.float32)),
        "bv": np.ascontiguousarray(np.asarray(bv, np.float32)),
    }
    in_maps = [
        {"x": x[c * B_PER_CORE:(c + 1) * B_PER_CORE], **common}
        for c in range(N_CORES)
    ]
    res = run_bass_kernel_spmd(nc, in_maps, list(range(N_CORES)), trace=_trace)
    outs = np.concatenate([res.results[c]["out"] for c in range(N_CORES)], axis=0)
    if _trace:
        kernel.last_exec_time_ns = res.exec_time_ns
        kernel.last_trace_info = (res.profile_json,
                                  (res.instructions_and_trace or (None, None))[1])
    return outs
